# revision 1
# baseline (speedup 1.0000x reference)
"""Multi-head self-attention (B=16,T=512,C=1024,H=16) on 8 NeuronCores.

Strategy: data-parallel over batch (2 batches/core), no collectives.
All matmuls run in float32r (full PE rate at moving-dim >= 256).
Layout is chosen so no on-device transposes are needed:
  - QK projection emits [f, tok] (q^T / k^T per head are direct slices)
  - V projection swaps matmul operands to emit v as [tok, f]
  - scores are computed transposed: sT[kt, qt]; softmax sums arrive via a
    ones-column appended to v in the AV matmul; masking is a 0/1 multiply
    after exp (exact, since exp>0 and rows are never fully masked).
  - normalization (1/l) is broadcast across partitions via a DRAM bounce
    and folded into the PSUM->SBUF copy of the attention output.
"""

import math

import numpy as np

import concourse.bass as bass
import concourse.mybir as mybir
import concourse.tile as tile
from concourse import bacc
from concourse.bass_utils import run_bass_kernel_spmd

N_CORES = 8
B, T, C = 16, 512, 1024
H = 16
DH = C // H  # 64
B_LOC = B // N_CORES  # 2
TOK = B_LOC * T  # 1024 tokens per core
P = 128
CT = C // P  # 8 contraction tiles
FQK = 2 * C  # q+k rows
DT = mybir.dt.float16
F32 = mybir.dt.float32


def _build_nc():
    nc = bacc.Bacc("TRN2", target_bir_lowering=False, debug=False,
                   num_devices=N_CORES)

    xT = nc.dram_tensor("xT", [C, TOK], DT, kind="ExternalInput").ap()
    wqkT = nc.dram_tensor("wqkT", [C, FQK], DT, kind="ExternalInput").ap()
    wvT = nc.dram_tensor("wvT", [C, C], DT, kind="ExternalInput").ap()
    woT = nc.dram_tensor("woT", [C, C], DT, kind="ExternalInput").ap()
    maskd = nc.dram_tensor("maskd", [T // P, P, P], DT,
                           kind="ExternalInput").ap()
    kpmb = nc.dram_tensor("kpmb", [B_LOC, T], F32, kind="ExternalInput").ap()
    bias = nc.dram_tensor("bias", [C], F32, kind="ExternalInput").ap()
    out = nc.dram_tensor("out", [TOK, C], F32, kind="ExternalOutput").ap()
    lall = nc.dram_tensor("lall", [B_LOC, H, T], F32).ap()
    linv_scr = nc.dram_tensor("linv_scr", [B_LOC, H, T], DT).ap()

    with tile.TileContext(nc) as tc:
        _emit(nc, tc, xT, wqkT, wvT, woT, maskd, kpmb, bias, out, lall,
              linv_scr)

    nc.compile()
    return nc


def _emit(nc, tc, xT, wqkT, wvT, woT, maskd, kpmb, bias, out, lall, linv_scr):
    from contextlib import ExitStack
    ctx = ExitStack()
    with ctx:
        singles = ctx.enter_context(tc.tile_pool(name="singles", bufs=1))
        wo_pool = ctx.enter_context(tc.tile_pool(name="wo", bufs=1))
        ps_a = ctx.enter_context(tc.tile_pool(name="ps_a", bufs=4, space="PSUM"))
        ps_s = ctx.enter_context(tc.tile_pool(name="ps_s", bufs=2, space="PSUM"))
        ps_o = ctx.enter_context(tc.tile_pool(name="ps_o", bufs=2, space="PSUM"))
        pt_pool = ctx.enter_context(tc.tile_pool(name="pt", bufs=2))
        linv_pool = ctx.enter_context(tc.tile_pool(name="linv", bufs=2))
        ao_stage_pool = ctx.enter_context(tc.tile_pool(name="aost", bufs=2))
        y_pool = ctx.enter_context(tc.tile_pool(name="y", bufs=3))

        NR = T // P  # 4 kt blocks

        # --- persistent SBUF tensors ---
        qk_sb = singles.tile([P, 16, TOK], DT)             # 32 KB/part
        v_sb = singles.tile([P, TOK // P, H, DH + 1], DT)  # 16.6 KB/part
        ao_b = [singles.tile([P, CT, T], DT, name=f"ao_b{b}")
                for b in range(B_LOC)]                     # 2x 8 KB/part

        bias_sb = singles.tile([P, C], F32)                # 4 KB/part
        maskd_sb = singles.tile([P, NR, P], DT)            # 1 KB/part
        kpmb_sb = singles.tile([P, B_LOC * NR], F32)

        with tc.tile_pool(name="xp", bufs=1) as x_pool, \
                tc.tile_pool(name="wq", bufs=3) as wq_pool, \
                tc.tile_pool(name="wv", bufs=1) as wv_pool:
            xk = [x_pool.tile([P, TOK], DT, tag=f"x_{k}", name=f"x_{k}")
                  for k in range(CT)]
            # first x chunk + first weight tile up front so the PE can start
            nc.sync.dma_start(out=xk[0][:], in_=xT[0:P, :])
            wq0 = wq_pool.tile([P, CT, P], DT, tag="wq", name="wq_0")
            nc.sync.dma_start(
                out=wq0[:],
                in_=wqkT[:, 0:P].rearrange("(k p) f -> p k f", p=P))
            for k in range(1, CT):
                nc.sync.dma_start(out=xk[k][:],
                                  in_=xT[k * P:(k + 1) * P, :])
            nc.sync.dma_start(out=maskd_sb[:],
                              in_=maskd.rearrange("r p q -> p r q"))
            nc.sync.dma_start(out=kpmb_sb[:],
                              in_=kpmb.rearrange("b (r p) -> p (b r)", p=P))

            # --- phase 1: QK projection -> qk_sb[f, tok] ---
            for j in range(16):
                if j == 0:
                    wq = wq0
                else:
                    wq = wq_pool.tile([P, CT, P], DT, tag="wq",
                                      name=f"wq_{j}")
                    nc.sync.dma_start(
                        out=wq[:],
                        in_=wqkT[:, j * P:(j + 1) * P].rearrange(
                            "(k p) f -> p k f", p=P))
                ps = [ps_a.tile([P, 512], F32, tag="ps_a", name=f"ps_qk_{j}_{tt}")
                      for tt in range(2)]
                for k in range(CT):
                    for tt in range(2):
                        nc.tensor.matmul(ps[tt][:], wq[:, k, :],
                                         xk[k][:, tt * 512:(tt + 1) * 512],
                                         start=(k == 0), stop=(k == CT - 1))
                for tt in range(2):
                    nc.vector.tensor_copy(
                        out=qk_sb[:, j, tt * 512:(tt + 1) * 512],
                        in_=ps[tt][:])

            # --- phase 2: V projection -> v_sb[tok, h, d] (+ ones col) ---
            nc.vector.memset(v_sb[:, :, :, DH:DH + 1], 1.0)
            for n in range(2):
                wv = [wv_pool.tile([P, 512], DT, tag=f"wv_{k}",
                                   name=f"wv_{n}_{k}") for k in range(CT)]
                for k in range(CT):
                    nc.sync.dma_start(
                        out=wv[k][:],
                        in_=wvT[k * P:(k + 1) * P, n * 512:(n + 1) * 512])
                for m in range(TOK // P):
                    ps = ps_a.tile([P, 512], F32, tag="ps_a", name=f"ps_v_{n}_{m}")
                    for k in range(CT):
                        nc.tensor.matmul(
                            ps[:], xk[k][:, m * P:(m + 1) * P], wv[k][:],
                            start=(k == 0), stop=(k == CT - 1))
                    nc.vector.tensor_copy(
                        out=v_sb[:, m, 8 * n:8 * n + 8, 0:DH],
                        in_=ps[:].rearrange("p (h d) -> p h d", d=DH))

        # out-projection weights prefetched here so they don't delay the
        # projection phase DMAs
        bias_bcast = bass.AP(tensor=bias.tensor, offset=bias.offset,
                             ap=[[0, P], *bias.ap])
        nc.gpsimd.dma_start(out=bias_sb[:], in_=bias_bcast)
        wo = [wo_pool.tile([P, 512], DT, tag=f"wo_{n}_{k}", name=f"wo_{n}_{k}")
              for n in range(2) for k in range(CT)]
        for n in range(2):
            for k in range(CT):
                nc.sync.dma_start(
                    out=wo[n * CT + k][:],
                    in_=woT[k * P:(k + 1) * P, n * 512:(n + 1) * 512])

        def yproj_chunk(b, i):
            n, m = i // (T // P), i % (T // P)
            ps = ps_a.tile([P, 512], F32, tag="ps_a", name=f"ps_y_{b}_{n}_{m}")
            for k in range(CT):
                nc.tensor.matmul(ps[:], ao_b[b][:, k, m * P:(m + 1) * P],
                                 wo[n * CT + k][:],
                                 start=(k == 0), stop=(k == CT - 1))
            y = y_pool.tile([P, 512], F32, tag="y")
            nc.vector.tensor_add(out=y[:], in0=ps[:],
                                 in1=bias_sb[:, n * 512:(n + 1) * 512])
            nc.sync.dma_start(
                out=out[b * T + m * P: b * T + (m + 1) * P,
                        n * 512:(n + 1) * 512],
                in_=y[:])

        # --- phase 3+4 interleaved per batch: b0 attention, then b1
        # attention with b0's out-projection chunks woven between heads ---
        for b in range(B_LOC):
            for h in range(H):
                jq, jk, dlo = h // 2, 8 + h // 2, DH * (h % 2)
                pT = pt_pool.tile([P, NR, 512], DT, tag="pT")
                sT = [ps_s.tile([P, 512], F32, tag="sT", name=f"sT_{b}_{h}_{r}")
                      for r in range(NR)]
                for r in range(NR):
                    kT = qk_sb[dlo:dlo + DH, jk,
                               b * T + r * P: b * T + (r + 1) * P]
                    qTr = qk_sb[dlo:dlo + DH, jq,
                                b * T + r * P:(b + 1) * T]
                    nc.tensor.matmul(sT[r][:, r * P:], kT, qTr,
                                     start=True, stop=True)
                    # exp over the un-masked tail; key-padding enters as an
                    # additive bias (0 or -1e30) per kt partition
                    nc.scalar.activation(
                        out=pT[:, r, r * P:], in_=sT[r][:, r * P:],
                        func=mybir.ActivationFunctionType.Exp,
                        bias=kpmb_sb[:, b * NR + r: b * NR + r + 1])
                    # causal mask inside the diagonal block only
                    nc.vector.tensor_mul(
                        out=pT[:, r, r * P:(r + 1) * P],
                        in0=pT[:, r, r * P:(r + 1) * P],
                        in1=maskd_sb[:, r, :])
                po = ps_o.tile([P, 512], F32, tag="po")
                for r in range(NR):
                    # masked columns of pT are never read: slice rhs/out
                    nc.tensor.matmul(po[0:DH + 1, r * P:],
                                     v_sb[:, b * NR + r, h, :],
                                     pT[:, r, r * P:],
                                     start=(r == 0), stop=(r == NR - 1))
                # stash row sums; normalization is batched per b
                lrow = linv_pool.tile([P, 512], F32, tag="lrow")
                nc.vector.tensor_copy(out=lrow[DH:DH + 1, :],
                                      in_=po[DH:DH + 1, :])
                nc.sync.dma_start(out=lall[b, h, :], in_=lrow[DH:DH + 1, :])
                if dlo == 0:
                    nc.vector.tensor_copy(
                        out=ao_b[b][0:DH, jq, :], in_=po[0:DH, :])
                else:
                    ao_st = ao_stage_pool.tile([DH, 512], DT, tag="ao_st")
                    nc.vector.tensor_copy(out=ao_st[:], in_=po[0:DH, :])
                    nc.sync.dma_start(out=ao_b[b][dlo:dlo + DH, jq, :],
                                      in_=ao_st[:])
                if b == 1 and h % 2 == 1:
                    yproj_chunk(0, h // 2)
                if h % (H // 2) == H // 2 - 1:
                    # 1/l for the finished half of the heads, then in-place
                    # normalize the corresponding ao c-tiles. The very last
                    # half is additionally sliced by qt-quarter so the final
                    # out-projection chunks (keyed by m = qt quarter) unblock
                    # incrementally instead of waiting for the whole chain.
                    half = h // (H // 2)
                    hs = slice(half * (H // 2), (half + 1) * (H // 2))
                    qslices = [slice(0, T)]
                    lpart = linv_pool.tile([H // 2, T], F32, tag="lpart",
                                           name=f"lpart_{b}_{half}")
                    nc.sync.dma_start(out=lpart[:], in_=lall[b, hs])
                    lpartd = linv_pool.tile([H // 2, T], DT, tag="lpartd",
                                            name=f"lpartd_{b}_{half}")
                    for qs in qslices:
                        nc.vector.reciprocal(out=lpart[:, qs],
                                             in_=lpart[:, qs])
                        nc.vector.tensor_copy(out=lpartd[:, qs],
                                              in_=lpart[:, qs])
                        nc.sync.dma_start(out=linv_scr[b, hs, qs],
                                          in_=lpartd[:, qs])
                        qlen = qs.stop - qs.start
                        for k in range(half * (CT // 2),
                                       (half + 1) * (CT // 2)):
                            lf = linv_pool.tile([P, T], DT, tag="lf")
                            for hf in range(2):
                                hh = 2 * k + hf
                                src_ap = bass.AP(
                                    tensor=linv_scr.tensor,
                                    offset=(linv_scr.offset
                                            + (b * H + hh) * T + qs.start),
                                    ap=[[0, DH], [1, qlen]])
                                nc.sync.dma_start(
                                    out=lf[hf * DH:(hf + 1) * DH, 0:qlen],
                                    in_=src_ap)
                            nc.vector.tensor_mul(
                                out=ao_b[b][:, k, qs],
                                in0=ao_b[b][:, k, qs],
                                in1=lf[:, 0:qlen])

        for i in range(2 * (T // P)):
            yproj_chunk(1, i)





_NC_CACHE = None


def _get_nc():
    global _NC_CACHE
    if _NC_CACHE is None:
        _NC_CACHE = _build_nc()
    return _NC_CACHE


def _prep_core_inputs(x, mask, key_padding_mask, w_qkv, w_out, b_out):
    """Host-side sharding + layout prep. Returns list of per-core in_maps."""
    x = np.asarray(x, dtype=np.float32)
    mask = np.asarray(mask)
    kpm = np.asarray(key_padding_mask)
    w_qkv = np.asarray(w_qkv, dtype=np.float32)
    w_out = np.asarray(w_out, dtype=np.float32)
    b_out = np.asarray(b_out, dtype=np.float32)

    scale = 1.0 / math.sqrt(DH)
    wqkT = w_qkv[:FQK].T.copy()  # [C, 2C]
    wqkT[:, :C] *= scale  # fold 1/sqrt(dh) into the Q weights
    wqkT = wqkT.astype(np.float16)
    wvT = np.ascontiguousarray(w_qkv[FQK:].T.astype(np.float16))  # [C, C]
    woT = np.ascontiguousarray(w_out.T.astype(np.float16))        # [C, C]

    # The kernel exploits the causal structure: it only applies mask values
    # inside the diagonal 128x128 blocks and zero-fills fully-masked blocks.
    # Verify the input mask really is lower-triangular.
    NRl = T // P
    exp_tril = np.tril(np.ones((T, T), dtype=mask.dtype))
    assert np.array_equal(mask, exp_tril), "kernel assumes causal tril mask"
    maskTf = mask.T.astype(np.float16)  # [kt, qt]
    maskd = np.stack([maskTf[r * P:(r + 1) * P, r * P:(r + 1) * P]
                      for r in range(NRl)])  # [NR, P, P]

    in_maps = []
    for i in range(N_CORES):
        xs = x[i * B_LOC:(i + 1) * B_LOC]      # [B_LOC, T, C]
        xT = np.ascontiguousarray(xs.reshape(TOK, C).T.astype(np.float16))
        kb = np.where(kpm[i * B_LOC:(i + 1) * B_LOC], -1e30,
                      0.0).astype(np.float32)  # [B_LOC, T]
        in_maps.append({
            "xT": xT,
            "wqkT": wqkT,
            "wvT": wvT,
            "woT": woT,
            "maskd": np.ascontiguousarray(maskd),
            "kpmb": kb,
            "bias": b_out,
        })
    return in_maps


def kernel(x, mask, key_padding_mask, w_qkv, w_out, b_out, _trace=False,
           _tmpdir=None):
    nc = _get_nc()
    in_maps = _prep_core_inputs(x, mask, key_padding_mask, w_qkv, w_out, b_out)
    res = run_bass_kernel_spmd(nc, in_maps, list(range(N_CORES)),
                               trace=_trace, tmpdir=_tmpdir)
    outs = [res.results[i]["out"].reshape(B_LOC, T, C) for i in range(N_CORES)]
    full = np.concatenate(outs, axis=0).astype(np.float32)
    kernel._last_exec_time_ns = res.exec_time_ns
    return full



# revision 12
# speedup vs baseline: 1.2800x; 1.2800x over previous
"""Multi-head self-attention (B=16,T=512,C=1024,H=16) on 8 NeuronCores.

Strategy: data-parallel over batch (2 batches/core), no collectives.
All GEMMs run in fp16 (fp8 fails the accuracy gate: quantization noise on
any of the three projection paths exceeds 2e-2 max-rel-err).

v3 design (vs v1 baseline):
  - Software pipelining: projection / out-projection matmuls are woven
    between attention matmul bursts as PE "filler", so the PE never idles
    while the Scalar engine computes exp and DVE/GpSimd evacuate psum and
    apply masks.  This also keeps the PE HAM clock at 8/8.
  - All input tensors are pre-arranged on the host into the exact SBUF
    layout so every DMA moves fat contiguous per-partition chunks
    (the v1 rearranging DMAs moved 128B..2B descriptors and serialized
    the queues for ~100us).
  - exp is ONE activation per head over the whole [128, 4x512] PSUM score
    tile (stale-garbage columns are exp'd too but never read by the
    r-sliced AV matmuls).  Causal masking inside the diagonal 128x128
    blocks is one strided GpSimd multiply per head.
  - key-padding: the V evacuation scale (per-partition kpm01) zeroes
    padded key rows of v; the ones-column of v (which produces the
    softmax denominators during the AV matmul) is multiplied by kpm01.
  - softmax normalization: l rows ride along in the single per-head
    psum evacuation, are inverted with reciprocal_approx_fast, broadcast
    across partitions with one DRAM-bounce DMA per (batch, half), and
    multiplied into ao on GpSimd.
"""

import math

import numpy as np

import concourse.bass as bass
import concourse.mybir as mybir
import concourse.tile as tile
from concourse import bacc
from concourse.bass_utils import run_bass_kernel_spmd

N_CORES = 8
B, T, C = 16, 512, 1024
H = 16
DH = C // H  # 64
B_LOC = B // N_CORES  # 2
TOK = B_LOC * T  # 1024 tokens per core
P = 128
CT = C // P  # 8 contraction tiles
NR = T // P  # 4 kt blocks
F16 = mybir.dt.float16
F32 = mybir.dt.float32

DEBUG = False


def _build_nc():
    nc = bacc.Bacc("TRN2", target_bir_lowering=False, debug=False,
                   num_devices=N_CORES)

    # all host-side pre-arranged to SBUF layout (fat contiguous DMAs)
    xd = nc.dram_tensor("xd", [P, CT, TOK], F16, kind="ExternalInput").ap()
    wqkd = nc.dram_tensor("wqkd", [P, H, CT, P], F16,
                          kind="ExternalInput").ap()
    wvd = nc.dram_tensor("wvd", [P, 2, CT, 512], F16,
                         kind="ExternalInput").ap()
    wod = nc.dram_tensor("wod", [P, 2, CT, 512], F16,
                         kind="ExternalInput").ap()
    maskd = nc.dram_tensor("maskd", [P, NR, P], F16,
                           kind="ExternalInput").ap()
    kpmvd = nc.dram_tensor("kpmvd", [P, 2 * NR], F16,
                           kind="ExternalInput").ap()  # 0/1 keep, per m
    kpmsd = nc.dram_tensor("kpmsd", [P, 2 * NR], F32,
                           kind="ExternalInput").ap()  # keep as f32 scale
    biasd = nc.dram_tensor("biasd", [C], F32, kind="ExternalInput").ap()
    out = nc.dram_tensor("out", [TOK, C], F32, kind="ExternalOutput").ap()
    # DRAM bounce buffer for broadcasting 1/l across partitions
    lbounce = nc.dram_tensor("lbounce", [B_LOC, 2, H // 2, T], F16).ap()

    with tile.TileContext(nc) as tc:
        _emit(nc, tc, xd, wqkd, wvd, wod, maskd, kpmvd, kpmsd, biasd,
              out, lbounce)

    nc.compile()
    return nc


def _emit(nc, tc, xd, wqkd, wvd, wod, maskd, kpmvd, kpmsd, biasd, out,
          lbounce):
    from contextlib import ExitStack
    ctx = ExitStack()
    with ctx:
        singles = ctx.enter_context(tc.tile_pool(name="singles", bufs=1))
        ps_proj = ctx.enter_context(
            tc.tile_pool(name="ps_proj", bufs=2, space="PSUM"))
        ps_s = ctx.enter_context(
            tc.tile_pool(name="ps_s", bufs=1, space="PSUM"))
        ps_o = ctx.enter_context(
            tc.tile_pool(name="ps_o", bufs=2, space="PSUM"))
        pt_pool = ctx.enter_context(tc.tile_pool(name="pt", bufs=2))
        lin_pool = ctx.enter_context(tc.tile_pool(name="lin", bufs=2))
        lf_pool = ctx.enter_context(tc.tile_pool(name="lf", bufs=2))
        ao_st_pool = ctx.enter_context(tc.tile_pool(name="aost", bufs=2))
        y_pool = ctx.enter_context(tc.tile_pool(name="y", bufs=3))

        # --- persistent SBUF tensors ---
        x_sb = singles.tile([P, CT, TOK], F16)        # 16 KB/part
        wqk_sb = singles.tile([P, H, CT, P], F16)     # 32 KB/part
        wv_sb = singles.tile([P, 2, CT, 512], F16)    # 16 KB/part
        wo_sb = singles.tile([P, 2, CT, 512], F16)    # 16 KB/part
        qk_sb = singles.tile([P, H, TOK], F16)        # 32 KB/part
        v_sb = singles.tile([P, TOK // P, H, DH + 1], F16)  # 16.6 KB/part
        ao_b = [singles.tile([P, CT, T], F16, name=f"ao_b{b}")
                for b in range(B_LOC)]                # 2x 8 KB/part
        bias_sb = singles.tile([P, C], F32)           # 4 KB/part
        maskd_sb = singles.tile([P, NR, P], F16)      # 1 KB/part
        kpmv_sb = singles.tile([P, 2 * NR], F16)
        kpms_sb = singles.tile([P, 2 * NR], F32)

        # --- prologue DMAs (fat, contiguous per partition) ---
        nc.sync.dma_start(out=x_sb[:, 0:CT // 2, :], in_=xd[:, 0:CT // 2, :])
        nc.sync.dma_start(out=x_sb[:, CT // 2:, :], in_=xd[:, CT // 2:, :])

        def dma_wqk(j):
            nc.sync.dma_start(out=wqk_sb[:, j], in_=wqkd[:, j])

        dma_wqk(0)
        dma_wqk(8)
        nc.sync.dma_start(out=wv_sb[:, 0], in_=wvd[:, 0])
        nc.sync.dma_start(out=maskd_sb[:], in_=maskd[:])
        nc.sync.dma_start(out=kpmv_sb[:], in_=kpmvd[:])
        nc.sync.dma_start(out=kpms_sb[:], in_=kpmsd[:])

        # ones-column of v = kpm01 (memset + broadcast multiply)
        ones_col = v_sb[:, :, :, DH:DH + 1]
        nc.gpsimd.memset(ones_col, 1.0)
        kpm_b = bass.AP(tensor=kpmv_sb.tensor, offset=kpmv_sb[:].offset,
                        ap=[kpmv_sb[:].ap[0], [1, 2 * NR], [0, H]])
        oc3 = bass.AP(tensor=v_sb.tensor, offset=ones_col.offset,
                      ap=[ones_col.ap[0], [H * (DH + 1), 2 * NR],
                          [DH + 1, H]])
        nc.vector.tensor_mul(out=oc3, in0=oc3, in1=kpm_b)

        # --- emitters ---
        def qk_j(j):
            for tt in range(2):
                ps = ps_proj.tile([P, 512], F32, tag="ps_proj",
                                  name=f"ps_qk_{j}_{tt}")
                for k in range(CT):
                    nc.tensor.matmul(
                        ps[:], wqk_sb[:, j, k, :],
                        x_sb[:, k, tt * 512:(tt + 1) * 512],
                        start=(k == 0), stop=(k == CT - 1))
                nc.vector.tensor_copy(
                    out=qk_sb[:, j, tt * 512:(tt + 1) * 512], in_=ps[:])

        def v_nm(n, m):
            ps = ps_proj.tile([P, 512], F32, tag="ps_proj",
                              name=f"ps_v_{n}_{m}")
            for k in range(CT):
                nc.tensor.matmul(
                    ps[:], x_sb[:, k, m * P:(m + 1) * P], wv_sb[:, n, k, :],
                    start=(k == 0), stop=(k == CT - 1))
            nc.vector.tensor_scalar_mul(
                out=v_sb[:, m, 8 * n:8 * n + 8, 0:DH],
                in0=ps[:].rearrange("p (h d) -> p h d", d=DH),
                scalar1=kpms_sb[:, m:m + 1])

        def scores(b, h):
            jq, jk, dlo = h // 2, 8 + h // 2, DH * (h % 2)
            sT4 = ps_s.tile([P, NR, 512], F32, tag="sT4",
                            name=f"sT4_{b}_{h}")
            for r in range(NR):
                kT = qk_sb[dlo:dlo + DH, jk,
                           b * T + r * P: b * T + (r + 1) * P]
                qTr = qk_sb[dlo:dlo + DH, jq, b * T + r * P:(b + 1) * T]
                nc.tensor.matmul(sT4[:, r, r * P:], kT, qTr,
                                 start=True, stop=True)
            return sT4

        def exp_mask(b, h, sT4):
            pT = pt_pool.tile([P, NR, 512], F16, tag="pT")
            nc.scalar.activation(out=pT[:], in_=sT4[:],
                                 func=mybir.ActivationFunctionType.Exp)
            base = pT[:, 0, 0:P]
            diag = bass.AP(tensor=base.tensor, offset=base.offset,
                           ap=[base.ap[0], [512 + P, NR], [1, P]])
            nc.gpsimd.tensor_mul(out=diag, in0=diag, in1=maskd_sb[:])
            return pT

        def av(b, h, pT):
            po = ps_o.tile([P, 512], F32, tag="po", name=f"po_{b}_{h}")
            for r in range(NR):
                nc.tensor.matmul(po[0:DH + 1, r * P:],
                                 v_sb[:, b * NR + r, h, :],
                                 pT[:, r, r * P:],
                                 start=(r == 0), stop=(r == NR - 1))
            return po

        lpart = {}

        def finish_head(b, h, po, lpart_t):
            # one evacuation of attention-out rows + the l row (row DH),
            # then SBUF->SBUF DMAs place them (DMA can shift partitions)
            ao_st = ao_st_pool.tile([DH + 1, 512], F16, tag="ao_st")
            if h % 2 == 0:
                nc.vector.tensor_copy(out=ao_st[:], in_=po[0:DH + 1, :])
            else:
                nc.scalar.activation(out=ao_st[:], in_=po[0:DH + 1, :],
                                     func=mybir.ActivationFunctionType.Copy)
            k, dlo = h // 2, DH * (h % 2)
            nc.sync.dma_start(out=ao_b[b][dlo:dlo + DH, k, :],
                              in_=ao_st[0:DH, :])
            nc.sync.dma_start(out=lpart_t[h % 8:h % 8 + 1, :],
                              in_=ao_st[DH:DH + 1, :])

        def attn_pair(b, p, filler1, filler2):
            hA, hB = 2 * p, 2 * p + 1
            key = (b, p // 4)
            if key not in lpart:
                lpart[key] = lin_pool.tile([H // 2, T], F16, tag="lpart",
                                           name=f"lpart_{key[0]}_{key[1]}")
            lp = lpart[key]
            sA = scores(b, hA)
            pA = exp_mask(b, hA, sA)
            for f in filler1:
                f()
            sB = scores(b, hB)
            pB = exp_mask(b, hB, sB)
            poA = av(b, hA, pA)
            finish_head(b, hA, poA, lp)
            for f in filler2:
                f()
            poB = av(b, hB, pB)
            finish_head(b, hB, poB, lp)

        def norm_half(b, half):
            lp = lpart[(b, half)]
            lp32 = lin_pool.tile([H // 2, T], F32, tag="lp32",
                                 name=f"lp32_{b}_{half}")
            nc.vector.tensor_copy(out=lp32[:], in_=lp[:])
            linv = lin_pool.tile([H // 2, T], F32, tag="linv",
                                 name=f"linv_{b}_{half}")
            nc.vector.reciprocal_approx_fast(out=linv[:], in_=lp32[:])
            linv16 = lin_pool.tile([H // 2, T], F16, tag="linv16",
                                   name=f"linv16_{b}_{half}")
            nc.vector.tensor_copy(out=linv16[:], in_=linv[:])
            nc.sync.dma_start(out=lbounce[b, half], in_=linv16[:])
            lf = lf_pool.tile([P, NR, 512], F16, tag="lf",
                              name=f"lf_{b}_{half}")
            boff = lbounce.offset + (b * 2 + half) * (H // 2) * T
            for ph in range(2):  # even heads -> parts 0-63, odd -> 64-127
                src = bass.AP(tensor=lbounce.tensor, offset=boff + ph * T,
                              ap=[[0, DH], [2 * T, NR], [1, T]])
                nc.sync.dma_start(out=lf[ph * DH:(ph + 1) * DH, :, :],
                                  in_=src)
            ks = slice(half * NR, (half + 1) * NR)
            nc.gpsimd.tensor_mul(out=ao_b[b][:, ks, :],
                                 in0=ao_b[b][:, ks, :], in1=lf[:])

        def yproj(b, i):
            n, m = i // NR, i % NR
            ps = ps_proj.tile([P, 512], F32, tag="ps_proj",
                              name=f"ps_y_{b}_{n}_{m}")
            for k in range(CT):
                nc.tensor.matmul(
                    ps[:], ao_b[b][:, k, m * P:(m + 1) * P],
                    wo_sb[:, n, k, :],
                    start=(k == 0), stop=(k == CT - 1))
            y = y_pool.tile([P, 512], F32, tag="y")
            nc.vector.tensor_add(out=y[:], in0=ps[:],
                                 in1=bias_sb[:, n * 512:(n + 1) * 512])
            nc.sync.dma_start(
                out=out[b * T + m * P: b * T + (m + 1) * P,
                        n * 512:(n + 1) * 512],
                in_=y[:])

        def dma_wo_bias():
            bias_bcast = bass.AP(tensor=biasd.tensor, offset=biasd.offset,
                                 ap=[[0, P], *biasd.ap])
            nc.gpsimd.dma_start(out=bias_sb[:], in_=bias_bcast)
            for n in range(2):
                nc.sync.dma_start(out=wo_sb[:, n], in_=wod[:, n])

        # --- emission schedule ---
        qk_j(0)
        qk_j(8)
        dma_wqk(1)
        dma_wqk(9)
        for m in range(4):
            v_nm(0, m)

        fill_b0 = {
            0: ([lambda: dma_wqk(2), lambda: dma_wqk(10),
                 lambda: qk_j(1), lambda: qk_j(9)], [lambda: v_nm(1, 0)]),
            1: ([lambda: dma_wqk(3), lambda: dma_wqk(11),
                 lambda: qk_j(2), lambda: qk_j(10)], [lambda: v_nm(1, 1)]),
            2: ([lambda: dma_wqk(4), lambda: dma_wqk(12),
                 lambda: qk_j(3), lambda: qk_j(11)], [lambda: v_nm(1, 2)]),
            3: ([lambda: dma_wqk(5), lambda: dma_wqk(13),
                 lambda: qk_j(4), lambda: qk_j(12)],
                [lambda: v_nm(1, 3), dma_wo_bias]),
            4: ([lambda: dma_wqk(6), lambda: dma_wqk(14),
                 lambda: qk_j(5), lambda: qk_j(13)], [lambda: v_nm(0, 4)]),
            5: ([lambda: dma_wqk(7), lambda: dma_wqk(15),
                 lambda: qk_j(6), lambda: qk_j(14)], [lambda: v_nm(0, 5)]),
            6: ([lambda: qk_j(7), lambda: qk_j(15)], [lambda: v_nm(0, 6)]),
            7: ([lambda: v_nm(0, 7)], [lambda: v_nm(1, 4)]),
        }
        # wv n=1 needed from b0 pair 0's filler v_nm(1,0)
        nc.sync.dma_start(out=wv_sb[:, 1], in_=wvd[:, 1])
        for p in range(8):
            f1, f2 = fill_b0[p]
            attn_pair(0, p, f1, f2)
            if p == 3:
                norm_half(0, 0)
        norm_half(0, 1)

        fill_b1 = {
            0: ([lambda: v_nm(1, 5)], [lambda: v_nm(1, 6)]),
            1: ([lambda: v_nm(1, 7)], [lambda: yproj(0, 0)]),
            2: ([lambda: yproj(0, 1)], [lambda: yproj(0, 2)]),
            3: ([lambda: yproj(0, 3)], [lambda: yproj(0, 4)]),
            4: ([lambda: yproj(0, 5)], []),
            5: ([lambda: yproj(0, 6)], []),
            6: ([lambda: yproj(0, 7)], []),
            7: ([], []),
        }
        for p in range(8):
            f1, f2 = fill_b1[p]
            attn_pair(1, p, f1, f2)
            if p == 3:
                norm_half(1, 0)
        norm_half(1, 1)

        for i in range(2 * NR):
            yproj(1, i)

        if DEBUG:
            dq = nc.dram_tensor("dbg_qk", [P, H, TOK], F16,
                                kind="ExternalOutput").ap()
            dv = nc.dram_tensor("dbg_v", [P, TOK // P, H, DH + 1], F16,
                                kind="ExternalOutput").ap()
            da = nc.dram_tensor("dbg_ao", [B_LOC, P, CT, T], F16,
                                kind="ExternalOutput").ap()
            nc.sync.dma_start(out=dq[:], in_=qk_sb[:])
            nc.sync.dma_start(out=dv[:], in_=v_sb[:])
            for b in range(B_LOC):
                nc.sync.dma_start(out=da[b], in_=ao_b[b][:])


_NC_CACHE = None


def _get_nc():
    global _NC_CACHE
    if _NC_CACHE is None:
        _NC_CACHE = _build_nc()
    return _NC_CACHE


def _prep_core_inputs(x, mask, key_padding_mask, w_qkv, w_out, b_out):
    """Host-side sharding + layout prep. Returns list of per-core in_maps."""
    x = np.asarray(x, dtype=np.float32)
    mask = np.asarray(mask)
    kpm = np.asarray(key_padding_mask)
    w_qkv = np.asarray(w_qkv, dtype=np.float32)
    w_out = np.asarray(w_out, dtype=np.float32)
    b_out = np.asarray(b_out, dtype=np.float32)

    FQK = 2 * C
    wqkT = w_qkv[:FQK].T.copy()  # [C, 2C]
    wqkT[:, :C] *= 1.0 / math.sqrt(DH)  # fold 1/sqrt(dh) into Q weights
    # [P, j, k, f]
    wqkd = np.ascontiguousarray(
        wqkT.astype(np.float16).reshape(CT, P, H, P).transpose(1, 2, 0, 3))
    wvd = np.ascontiguousarray(
        w_qkv[FQK:].T.astype(np.float16).reshape(CT, P, 2, 512)
        .transpose(1, 2, 0, 3))
    wod = np.ascontiguousarray(
        w_out.T.astype(np.float16).reshape(CT, P, 2, 512)
        .transpose(1, 2, 0, 3))

    exp_tril = np.tril(np.ones((T, T), dtype=mask.dtype))
    assert np.array_equal(mask, exp_tril), "kernel assumes causal tril mask"
    maskTf = mask.T.astype(np.float16)  # [kt, qt]
    maskd = np.ascontiguousarray(
        np.stack([maskTf[r * P:(r + 1) * P, r * P:(r + 1) * P]
                  for r in range(NR)]).transpose(1, 0, 2))  # [P, NR, P]

    in_maps = []
    for i in range(N_CORES):
        xs = x[i * B_LOC:(i + 1) * B_LOC]      # [B_LOC, T, C]
        xdd = np.ascontiguousarray(
            xs.reshape(TOK, C).T.astype(np.float16)
            .reshape(CT, P, TOK).transpose(1, 0, 2))  # [P, k, t]
        keep = (~kpm[i * B_LOC:(i + 1) * B_LOC]).astype(np.float32)
        keep_pm = keep.reshape(2 * NR, P).T  # [P, m]
        in_maps.append({
            "xd": xdd,
            "wqkd": wqkd,
            "wvd": wvd,
            "wod": wod,
            "maskd": maskd,
            "kpmvd": np.ascontiguousarray(keep_pm.astype(np.float16)),
            "kpmsd": np.ascontiguousarray(keep_pm.astype(np.float32)),
            "biasd": b_out,
        })
    return in_maps


def kernel(x, mask, key_padding_mask, w_qkv, w_out, b_out, _trace=False,
           _tmpdir=None):
    nc = _get_nc()
    in_maps = _prep_core_inputs(x, mask, key_padding_mask, w_qkv, w_out,
                                b_out)
    res = run_bass_kernel_spmd(nc, in_maps, list(range(N_CORES)),
                               trace=_trace, tmpdir=_tmpdir)
    outs = [res.results[i]["out"].reshape(B_LOC, T, C)
            for i in range(N_CORES)]
    full = np.concatenate(outs, axis=0).astype(np.float32)
    kernel._last_exec_time_ns = res.exec_time_ns
    return full
